# revision 10
# baseline (speedup 1.0000x reference)
"""Per-sample depthwise 7x7 SAME cross-correlation on 8 trn2 NeuronCores.

Problem: inputs [32,128,128,128] (B,H,W,C), kernels [32,7,7,128] (B,KH,KW,C).
out[b,y,x,c] = sum_{i,j} inputs[b, y+i-3, x+j-3, c] * kernels[b,i,j,c]

Strategy (pure data parallel, 4 samples/core, TensorEngine formulation):
  For one (b, c) channel image X [y', x] the 2D conv factors as 7 banded
  matmuls accumulated in PSUM:

      out[y, x] = sum_j  sum_{y'}  T_j[y', y] * X[y', x + j - 3]
      T_j[y', y] = w[y' - y + 3, j]   (7-diagonal banded Toeplitz)

  lhsT (stationary) = T_j (K=y'=128, M=y=128), rhs (moving) = the image
  read at free-dim offset j from an x-padded SBUF tile, N = x = 128.
  One PSUM region accumulates all 49 taps (7 matmuls x 7-wide band), so
  the PE does 7 fused MACs per streamed column-row vs 1/cycle/partition
  for any elementwise engine: 3584 matmuls/core ~= 191 us steady-state
  (cost model) vs 2.43 ms for the tuned 3-engine elementwise formulation.

  The per-(b,c,j) banded matrices are NOT materialized densely: their zero
  cells never change, so each rebuild only rewrites the band. With the K
  axis flipped (host stores image rows reversed; the flip cancels in the
  contraction) the band cell (p, y) reads w[130 - p - y]: each 16-column
  chunk of T is a rectangular [<=22, 16] block reading a tiny per-(b,c,j)
  DRAM buffer wr[u] = w[130-u] at u = p + y, an overlapping positive-
  stride affine AP. T tiles are c-minor ([128p, 128m, 7j, CBc]) and wr is
  [u, j, c]-ordered so one chunk write for a whole channel batch is one
  multi-KB-contiguous descriptor per partition on both sides (~13 MB of
  band traffic total vs 115 MB dense). The first use of each double
  buffer writes the full partition range (same buffers, zeros included);
  steady state rewrites bands only.

  Pipeline: double-buffered T/X/out with the next group's input DMAs
  emitted BEFORE this group's out-DMA (the in-order SP sequencer would
  otherwise head-of-line block them on the output copies and stall the
  PE). PSUM packs 4 channels per bank; VectorE drains PSUM->SBUF in
  512-wide bf16 copies.

  bf16 operands (PSUM accumulates fp32); rel err ~4.5e-3 << 2e-2 gate.

  Measured (2026-08-08, measure2.py steady-state call timing on the 8
  axon-tunneled trn2 cores): 315.9 us/pass steady state, ~370-396 us
  single pass (TimelineSim 238.0 us; ratio 1.66 = exposed LDWEIGHTS ~24
  ns/MM per mmbench.py). Previous elementwise kernel: 2433.6 us model,
  ~3164 us reported -> ~8x real speedup. mmbench.py also showed strided
  (c-minor) weight APs cost the same as contiguous ones, so the
  DMA-friendly T layout is free.
"""

import numpy as np
import ml_dtypes

import concourse.bass as bass
import concourse.tile as tile
from concourse import bacc, mybir
from concourse.bass_utils import run_bass_kernel_spmd

B, H, W, C = 32, 128, 128, 128
KH = KW = 7
PAD = 3
N_CORES = 8
BPC = B // N_CORES  # samples per core
WP = W + 2 * PAD  # 134: x-padded width
CB = 32  # channels per group (pipeline stage)
N_GROUPS = C // CB
CC = 32  # T band-chunk column width
N_CHUNK = 128 // CC
WRLEN = 256  # per-(b,c,j) band buffer length (u = p + y in [0, 254])
QUAD = 8  # channels per PSUM bank

_PROGRAM_CACHE = {}


def _chunk_geometry(q, full):
    """Rows [P0, P0+R) of T covering band cells for columns [CC*q, CC*q+CC)."""
    if full:
        return 0, 128
    p0 = max(0, 124 - (CC * q + CC - 1))
    pend = min(127, 130 - CC * q)
    return p0, pend - p0 + 1


def _build_program(repeat=1):
    f32 = mybir.dt.float32
    bf16 = mybir.dt.bfloat16
    nc = bacc.Bacc("TRN2", target_bir_lowering=False, debug=False)
    # x: [b, y(flipped), c, x(padded)]; w: band buffers; o: [b, y, c, x]
    x_h = nc.dram_tensor("x", [BPC, H, C, WP], bf16, kind="ExternalInput")
    w_h = nc.dram_tensor(
        "w", [BPC, N_GROUPS, WRLEN, KW, CB], bf16, kind="ExternalInput"
    )
    o_h = nc.dram_tensor("o", [BPC, H, C, W], bf16, kind="ExternalOutput")
    x, o = x_h.ap(), o_h.ap()

    # wr strides (elements): [b, g, u, j, c]
    SW_U = KW * CB
    SW_G = WRLEN * SW_U
    SW_B = N_GROUPS * SW_G

    with tile.TileContext(nc) as tc:
        with (
            tc.tile_pool(name="xbuf", bufs=1) as xpool,
            tc.tile_pool(name="tbuf", bufs=1) as tpool,
            tc.tile_pool(name="obuf", bufs=1) as opool,
            tc.tile_pool(name="psum", bufs=4, space="PSUM") as psump,
        ):
            xb = [xpool.tile([128, CB, WP], bf16, name=f"xb{i}") for i in range(2)]
            # T super-tile, c-minor: [p=y'flip, m=y, j, c]
            tb = [tpool.tile([128, 128, KW, CB], bf16, name=f"tb{i}") for i in range(2)]
            ob = [opool.tile([128, CB, W], bf16, name=f"ob{i}") for i in range(2)]

            groups = [
                (b, g)
                for _ in range(repeat)
                for b in range(BPC)
                for g in range(N_GROUPS)
            ]
            n = len(groups)

            def emit_in_dmas(gi):
                """Input DMAs for group gi (X image batch + T band rewrites)."""
                b, g = groups[gi]
                par = gi % 2
                c0 = g * CB
                xt, tt = xb[par], tb[par]
                # input image batch: [y(128 part), c(CB), x(134)]
                nc.sync.dma_start(out=xt, in_=x[b, :, c0 : c0 + CB, :])
                # band rewrites: one DMA per column-chunk, all (c, j) at once.
                # First use of each buffer writes the full partition range so
                # the static zeros get initialized.
                full = gi < 2
                for q in range(N_CHUNK):
                    p0, r = _chunk_geometry(q, full)
                    dst = tt[p0 : p0 + r, CC * q : CC * q + CC, :, :]
                    src = bass.AP(
                        tensor=w_h,
                        offset=b * SW_B + g * SW_G + (p0 + CC * q) * SW_U,
                        ap=[[SW_U, r], [SW_U, CC], [CB, KW], [1, CB]],
                    )
                    eng = nc.sync if q % 2 == 0 else nc.scalar
                    eng.dma_start(out=dst, in_=src)

            # Software-pipelined emission: group g+2's input DMAs are issued
            # BEFORE group g's out-DMA on the same (in-order) SP sequencer.
            emit_in_dmas(0)
            if n > 1:
                emit_in_dmas(1)
            for gi, (b, g) in enumerate(groups):
                par = gi % 2
                c0 = g * CB
                xt, tt, ot = xb[par], tb[par], ob[par]

                for c4 in range(CB // QUAD):
                    pt = psump.tile([128, QUAD * W], f32, name="pt", tag="pt")
                    for k in range(QUAD):
                        ci = c4 * QUAD + k
                        for j in range(KW):
                            nc.tensor.matmul(
                                out=pt[:, k * W : (k + 1) * W],
                                lhsT=tt[:, :, j, ci],
                                rhs=xt[:, ci, j : j + W],
                                start=(j == 0),
                                stop=(j == KW - 1),
                            )
                    nc.vector.tensor_copy(
                        out=ot[:, c4 * QUAD : (c4 + 1) * QUAD, :], in_=pt
                    )

                if gi + 2 < n:
                    emit_in_dmas(gi + 2)
                nc.sync.dma_start(out=o[b, :, c0 : c0 + CB, :], in_=ot)

    nc.compile()
    return nc


def _get_program():
    if "nc" not in _PROGRAM_CACHE:
        _PROGRAM_CACHE["nc"] = _build_program()
    return _PROGRAM_CACHE["nc"]


def _prep_inputs(inputs, kernels):
    """Host-side shard + layout transform. Returns per-core input maps."""
    bf16 = ml_dtypes.bfloat16
    # [B,H,W,C] -> [b, y, c, x], y flipped, x padded to 134
    xt = np.zeros((B, H, C, WP), bf16)
    xt[:, :, :, PAD : PAD + W] = np.transpose(inputs[:, ::-1], (0, 1, 3, 2))
    # band buffers: wr[b, g, u, j, cg] = w[b, 130-u, j, g*CB+cg], u in [124,130]
    wr = np.zeros((B, N_GROUPS, WRLEN, KW, CB), bf16)
    kr = kernels.reshape(B, KH, KW, N_GROUPS, CB)
    wr[:, :, 124:131] = np.transpose(kr[:, ::-1], (0, 3, 1, 2, 4))
    in_maps = []
    for k in range(N_CORES):
        sl = slice(k * BPC, (k + 1) * BPC)
        in_maps.append({"x": xt[sl], "w": wr[sl]})
    return in_maps


def _gather_output(results):
    full = np.concatenate([r["o"] for r in results], axis=0)  # [B, y, c, x]
    return np.ascontiguousarray(
        np.transpose(full, (0, 1, 3, 2)).astype(np.float32)
    )


def run_spmd(inputs, kernels, **spmd_kwargs):
    """Run on all 8 cores; returns (output, BassKernelResults)."""
    nc = _get_program()
    in_maps = _prep_inputs(np.asarray(inputs), np.asarray(kernels))
    res = run_bass_kernel_spmd(nc, in_maps, list(range(N_CORES)), **spmd_kwargs)
    return _gather_output(res.results), res


def kernel(inputs, kernels):
    out, _ = run_spmd(inputs, kernels)
    return out


# revision 11
# speedup vs baseline: 1.0022x; 1.0022x over previous
"""Per-sample depthwise 7x7 SAME cross-correlation on 8 trn2 NeuronCores.

Problem: inputs [32,128,128,128] (B,H,W,C), kernels [32,7,7,128] (B,KH,KW,C).
out[b,y,x,c] = sum_{i,j} inputs[b, y+i-3, x+j-3, c] * kernels[b,i,j,c]

Strategy (pure data parallel, 4 samples/core, TensorEngine formulation):
  For one (b, c) channel image X [y', x] the 2D conv factors as 7 banded
  matmuls accumulated in PSUM:

      out[y, x] = sum_j  sum_{y'}  T_j[y', y] * X[y', x + j - 3]
      T_j[y', y] = w[y' - y + 3, j]   (7-diagonal banded Toeplitz)

  lhsT (stationary) = T_j (K=y'=128, M=y=128), rhs (moving) = the image
  read at free-dim offset j from an x-padded SBUF tile, N = x = 128.
  One PSUM region accumulates all 49 taps (7 matmuls x 7-wide band), so
  the PE does 7 fused MACs per streamed column-row vs 1/cycle/partition
  for any elementwise engine: 3584 matmuls/core ~= 191 us steady-state
  (cost model) vs 2.43 ms for the tuned 3-engine elementwise formulation.

  The per-(b,c,j) banded matrices are NOT materialized densely: their zero
  cells never change, so each rebuild only rewrites the band. With the K
  axis flipped (host stores image rows reversed; the flip cancels in the
  contraction) the band cell (p, y) reads w[130 - p - y]: each 16-column
  chunk of T is a rectangular [<=22, 16] block reading a tiny per-(b,c,j)
  DRAM buffer wr[u] = w[130-u] at u = p + y, an overlapping positive-
  stride affine AP. T tiles are c-minor ([128p, 128m, 7j, CBc]) and wr is
  [u, j, c]-ordered so one chunk write for a whole channel batch is one
  multi-KB-contiguous descriptor per partition on both sides (~13 MB of
  band traffic total vs 115 MB dense). The first use of each double
  buffer writes the full partition range (same buffers, zeros included);
  steady state rewrites bands only.

  Pipeline: double-buffered T/X/out with the next group's input DMAs
  emitted BEFORE this group's out-DMA (the in-order SP sequencer would
  otherwise head-of-line block them on the output copies and stall the
  PE). PSUM packs 4 channels per bank; VectorE drains PSUM->SBUF in
  512-wide bf16 copies.

  bf16 operands (PSUM accumulates fp32); rel err ~4.5e-3 << 2e-2 gate.

  Measured (2026-08-08, measure2.py steady-state call timing on the 8
  axon-tunneled trn2 cores): 315.9 us/pass steady state, ~370-396 us
  single pass (TimelineSim 238.0 us; ratio 1.66 = exposed LDWEIGHTS ~24
  ns/MM per mmbench.py). Previous elementwise kernel: 2433.6 us model,
  ~3164 us reported -> ~8x real speedup. mmbench.py also showed strided
  (c-minor) weight APs cost the same as contiguous ones, so the
  DMA-friendly T layout is free.
"""

import numpy as np
import ml_dtypes

import concourse.bass as bass
import concourse.tile as tile
from concourse import bacc, mybir
from concourse.bass_utils import run_bass_kernel_spmd

B, H, W, C = 32, 128, 128, 128
KH = KW = 7
PAD = 3
N_CORES = 8
BPC = B // N_CORES  # samples per core
WP = W + 2 * PAD  # 134: x-padded width
CB = 32  # channels per group (pipeline stage)
N_GROUPS = C // CB
CC = 16  # T band-chunk column width
N_CHUNK = 128 // CC
WRLEN = 256  # per-(b,c,j) band buffer length (u = p + y in [0, 254])
QUAD = 4  # channels per PSUM bank

_PROGRAM_CACHE = {}


def _chunk_geometry(q, full):
    """Rows [P0, P0+R) of T covering band cells for columns [CC*q, CC*q+CC)."""
    if full:
        return 0, 128
    p0 = max(0, 124 - (CC * q + CC - 1))
    pend = min(127, 130 - CC * q)
    return p0, pend - p0 + 1


def _build_program(repeat=1):
    f32 = mybir.dt.float32
    bf16 = mybir.dt.bfloat16
    nc = bacc.Bacc("TRN2", target_bir_lowering=False, debug=False)
    # x: [b, y(flipped), c, x(padded)]; w: band buffers; o: [b, y, c, x]
    x_h = nc.dram_tensor("x", [BPC, H, C, WP], bf16, kind="ExternalInput")
    w_h = nc.dram_tensor(
        "w", [BPC, N_GROUPS, WRLEN, KW, CB], bf16, kind="ExternalInput"
    )
    o_h = nc.dram_tensor("o", [BPC, H, C, W], bf16, kind="ExternalOutput")
    x, o = x_h.ap(), o_h.ap()

    # wr strides (elements): [b, g, u, j, c]
    SW_U = KW * CB
    SW_G = WRLEN * SW_U
    SW_B = N_GROUPS * SW_G

    with tile.TileContext(nc) as tc:
        with (
            tc.tile_pool(name="xbuf", bufs=1) as xpool,
            tc.tile_pool(name="tbuf", bufs=1) as tpool,
            tc.tile_pool(name="obuf", bufs=1) as opool,
            tc.tile_pool(name="psum", bufs=8, space="PSUM") as psump,
        ):
            xb = [xpool.tile([128, CB, WP], bf16, name=f"xb{i}") for i in range(2)]
            # T super-tile, c-minor: [p=y'flip, m=y, j, c]
            tb = [tpool.tile([128, 128, KW, CB], bf16, name=f"tb{i}") for i in range(2)]
            ob = [opool.tile([128, CB, W], bf16, name=f"ob{i}") for i in range(2)]

            groups = [
                (b, g)
                for _ in range(repeat)
                for b in range(BPC)
                for g in range(N_GROUPS)
            ]
            n = len(groups)

            def emit_in_dmas(gi):
                """Input DMAs for group gi (X image batch + T band rewrites)."""
                b, g = groups[gi]
                par = gi % 2
                c0 = g * CB
                xt, tt = xb[par], tb[par]
                # input image batch: [y(128 part), c(CB), x(134)]
                nc.sync.dma_start(out=xt, in_=x[b, :, c0 : c0 + CB, :])
                # band rewrites: one DMA per column-chunk, all (c, j) at once.
                # First use of each buffer writes the full partition range so
                # the static zeros get initialized.
                full = gi < 2
                for q in range(N_CHUNK):
                    p0, r = _chunk_geometry(q, full)
                    dst = tt[p0 : p0 + r, CC * q : CC * q + CC, :, :]
                    src = bass.AP(
                        tensor=w_h,
                        offset=b * SW_B + g * SW_G + (p0 + CC * q) * SW_U,
                        ap=[[SW_U, r], [SW_U, CC], [CB, KW], [1, CB]],
                    )
                    eng = nc.sync if q % 2 == 0 else nc.scalar
                    eng.dma_start(out=dst, in_=src)

            # Software-pipelined emission: group g+2's input DMAs are issued
            # BEFORE group g's out-DMA on the same (in-order) SP sequencer.
            emit_in_dmas(0)
            if n > 1:
                emit_in_dmas(1)
            for gi, (b, g) in enumerate(groups):
                par = gi % 2
                c0 = g * CB
                xt, tt, ot = xb[par], tb[par], ob[par]

                for c4 in range(CB // QUAD):
                    pt = psump.tile([128, QUAD * W], f32, name="pt", tag="pt")
                    for k in range(QUAD):
                        ci = c4 * QUAD + k
                        for j in range(KW):
                            nc.tensor.matmul(
                                out=pt[:, k * W : (k + 1) * W],
                                lhsT=tt[:, :, j, ci],
                                rhs=xt[:, ci, j : j + W],
                                start=(j == 0),
                                stop=(j == KW - 1),
                            )
                    nc.vector.tensor_copy(
                        out=ot[:, c4 * QUAD : (c4 + 1) * QUAD, :], in_=pt
                    )

                if gi + 2 < n:
                    emit_in_dmas(gi + 2)
                nc.sync.dma_start(out=o[b, :, c0 : c0 + CB, :], in_=ot)

    nc.compile()
    return nc


def _get_program():
    if "nc" not in _PROGRAM_CACHE:
        _PROGRAM_CACHE["nc"] = _build_program()
    return _PROGRAM_CACHE["nc"]


def _prep_inputs(inputs, kernels):
    """Host-side shard + layout transform. Returns per-core input maps."""
    bf16 = ml_dtypes.bfloat16
    # [B,H,W,C] -> [b, y, c, x], y flipped, x padded to 134
    xt = np.zeros((B, H, C, WP), bf16)
    xt[:, :, :, PAD : PAD + W] = np.transpose(inputs[:, ::-1], (0, 1, 3, 2))
    # band buffers: wr[b, g, u, j, cg] = w[b, 130-u, j, g*CB+cg], u in [124,130]
    wr = np.zeros((B, N_GROUPS, WRLEN, KW, CB), bf16)
    kr = kernels.reshape(B, KH, KW, N_GROUPS, CB)
    wr[:, :, 124:131] = np.transpose(kr[:, ::-1], (0, 3, 1, 2, 4))
    in_maps = []
    for k in range(N_CORES):
        sl = slice(k * BPC, (k + 1) * BPC)
        in_maps.append({"x": xt[sl], "w": wr[sl]})
    return in_maps


def _gather_output(results):
    full = np.concatenate([r["o"] for r in results], axis=0)  # [B, y, c, x]
    return np.ascontiguousarray(
        np.transpose(full, (0, 1, 3, 2)).astype(np.float32)
    )


def run_spmd(inputs, kernels, **spmd_kwargs):
    """Run on all 8 cores; returns (output, BassKernelResults)."""
    nc = _get_program()
    in_maps = _prep_inputs(np.asarray(inputs), np.asarray(kernels))
    res = run_bass_kernel_spmd(nc, in_maps, list(range(N_CORES)), **spmd_kwargs)
    return _gather_output(res.results), res


def kernel(inputs, kernels):
    out, _ = run_spmd(inputs, kernels)
    return out
